# revision 36
# baseline (speedup 1.0000x reference)
"""Trainium2 8-core kernel for nn_Attn_user_47863115547245.

reference:
    proj     = id_emb @ attn_W.T + attn_b                  # [seq, hid]
    energies = w1*(user @ proj.T) + w2*(socail @ proj.T)   # [state, seq]
    out      = softmax(energies, axis=-1)

Restructuring (exact up to float rounding):
  * linearity: energies = c @ W @ id.T with c = w1*user + w2*socail,
    computed on the host (c is the only state-side tensor uploaded).
  * (c @ W) first: state(2048) < seq(4096) makes this ordering cheaper.
  * the bias term is constant along the softmax axis -> cancels; dropped.

Sharding: data-parallel over state rows, 256 rows/core x 8 cores.
W and id_emb (fp16, pre-transposed/packed) replicated; softmax is
row-local -> zero collectives.

Schedule (from NTFF traces; see kernel body comments):
  * DMA queues: sequencers stall on DIRECT2D queue-depth backpressure,
    so the scalar ring carries only W evens up front (its engine must
    dispatch casts/exps on time) with the id even blocks woven into
    the mm2 emission stream; the sync ring (no engine work) carries
    cT + W odds + id odds + output.
  * warmup garbage matmuls bridge PE from t~7us to the first mm1 dep
    (the HAM clock gate re-throttles after long idle gaps).
  * mm1 single pass, h-arrival paced: h-outer accumulation into 4 PSUM
    banks (2 k-blocks packed per bank = one accumulation group), then
    4 bank copies spread over DVE+ACT so mm2's k-accumulation starts
    without serializing behind one engine queue.
  * mm2 per 512-col seq block, k-inner, 4 rotating PSUM banks; blocks
    s0..s4 alternate m0/m1 (stays just behind the id arrival stream),
    then m0 s5..7, finish(0) fully overlapped with m1 s5..7's matmuls.
  * quick-max softmax: per-row max of bank s0 only is the exp shift
    (gap to the true row max is ~O(10) for gaussian energies, well
    inside fp32 exp range), so there are no per-chunk MAX ops and no
    rescale bookkeeping. Unnormalized exp is held bf16 (fp32 exponent
    range; 16-bit data keeps TensorScalarPtr on the fast DVE uop -
    fp32 src there is ~20x slower); normalize is x(1/S) as a
    dual-AP-scalar tensor_scalar, fanned across DVE/GpSimd/ACT and
    chunk-pipelined with the output DMAs on both rings.
"""

import numpy as np

STATE, SEQ, HID = 2048, 4096, 1024
NCORES = 8
ROWS = STATE // NCORES        # 256 state rows per core
P = 128                       # partitions
KT = HID // P                 # 8 contraction tiles
MT = ROWS // P                # 2 output row tiles per core
SB = 512                      # seq block (one fp32 PSUM bank)
ST = SEQ // SB                # 8 seq blocks
WARMUP_MM = 12                # PE warmup matmuls (HAM un-throttle)

_graph_cache: dict = {}


def _build(*_ignored):
    """Build the per-core Bass graph (no runtime parameters)."""
    import concourse.bacc as bacc
    import concourse.mybir as mybir
    import concourse.bass as bass
    from concourse import tile

    f32, f16 = mybir.dt.float32, mybir.dt.float16
    bf16 = mybir.dt.bfloat16
    AX = mybir.AxisListType.X
    ALU = mybir.AluOpType
    ACTF = mybir.ActivationFunctionType

    nc = bacc.Bacc()

    cT = nc.declare_dram_parameter("cT", [P, KT, ROWS], f16, isOutput=False)
    Wp = nc.declare_dram_parameter("Wp", [P, KT, HID], f16, isOutput=False)
    idT = nc.declare_dram_parameter("idT", [ST, P, KT, SB], f16, isOutput=False)
    out = nc.declare_dram_parameter("out", [ROWS, SEQ], bf16, isOutput=True)

    with tile.TileContext(nc) as tc:
        with (
            tc.tile_pool(name="inp", bufs=1) as inp,
            tc.tile_pool(name="work", bufs=1) as work,
            tc.tile_pool(name="psum", bufs=1,
                         space=bass.MemorySpace.PSUM) as psp,
        ):
            # ---- PE warmup: garbage matmuls lift the HAM clock gate ----
            wgarb = work.tile([P, SB], f16, tag="warmgarb")
            nc.gpsimd.memset(wgarb[:], 0.0)
            # mm1 accumulator: 4 banks, each packing 2 k-blocks of 256
            ps1 = psp.tile([P, 4, 2, ROWS], f32, tag="mm1")
            for _ in range(WARMUP_MM):
                nc.tensor.matmul(
                    ps1[:, 0, 0, :], wgarb[:, :P], wgarb[:, :ROWS],
                    start=True, stop=True)

            # ---- input DMAs, alternating between the two HWDGE rings ----
            cT_sb = inp.tile([P, KT, ROWS], f16)
            W_sb = inp.tile([P, KT, HID], f16)
            id_sb = inp.tile([P, ST, KT, SB], f16)

            # Queue plan (two HWDGE rings, ~188GB/s each): a sequencer
            # stalls on DIRECT2D queue-depth backpressure but NOT on
            # dispatching engine ops into engine queues -- so scalar's
            # engine work (casts, exps) is emitted BEFORE its deep id
            # descriptor list, and the id descriptors are woven into the
            # mm2 emission stream below.
            #   scalar ring: W evens, then id even blocks (woven).
            #   sync ring (no engine work): cT quarters woven with W
            #   odds, then id odd blocks, then the output DMAs.
            # mm1 h-step h needs only W[h] + cT[h] -> h-paced from ~9us.
            H2 = KT // 2

            def id_dma(eng, s, quarters=False):
                # k-split pieces back-to-back on one ring: each piece's
                # completion sem frees its k-range of mm2 matmuls before
                # the whole block lands (quarters for the blocks the PE
                # is known to wait on).
                if quarters:
                    eng.dma_start(id_sb[:, s, 0:2, :], idT[s][:, 0:2, :])
                    eng.dma_start(id_sb[:, s, 2:4, :], idT[s][:, 2:4, :])
                else:
                    eng.dma_start(id_sb[:, s, :H2, :], idT[s][:, :H2, :])
                eng.dma_start(id_sb[:, s, H2:, :], idT[s][:, H2:, :])

            for q in range(4):
                nc.scalar.dma_start(W_sb[:, 2 * q, :], Wp[:, 2 * q, :])
                nc.sync.dma_start(
                    cT_sb[:, 2 * q:2 * q + 2, :], cT[:, 2 * q:2 * q + 2, :])
                nc.sync.dma_start(W_sb[:, 2 * q + 1, :], Wp[:, 2 * q + 1, :])
            for s in range(1, ST, 2):
                id_dma(nc.sync, s, quarters=True)

            # ---- mm1: tmpT[k,m] = sum_h W[h,k]*c[m,h], h-outer --------
            # Single W-arrival-paced pass over all 4 banks.
            # Each bank holds 2 packed k-blocks as ONE accumulation group
            # (start=True zeroes the whole 2KB bank: first write starts
            # the group, last write stops it, pending-zero bits zero each
            # region on first touch).
            tmpT_sb = work.tile([P, KT, ROWS], f16)

            for h in range(KT):
                for kb in range(KT):
                    nc.tensor.matmul(
                        ps1[:, kb // 2, kb % 2, :],
                        W_sb[:, h, P * kb:P * (kb + 1)],
                        cT_sb[:, h, :],
                        start=(h == 0 and kb % 2 == 0),
                        stop=(h == KT - 1 and kb % 2 == 1),
                    )
            # bank copies spread over 3 engines so mm2's k-accumulation
            # isn't serialized behind one DVE queue
            nc.vector.tensor_copy(tmpT_sb[:, 0:2, :], ps1[:, 0, :, :])
            nc.scalar.activation(
                tmpT_sb[:, 2:4, :], ps1[:, 1, :, :], ACTF.Copy)
            nc.scalar.activation(
                tmpT_sb[:, 4:6, :], ps1[:, 2, :, :], ACTF.Copy)
            nc.vector.tensor_copy(tmpT_sb[:, 6:8, :], ps1[:, 3, :, :])

            # ---- mm2 (per seq block, k-inner) + quick-max softmax ----
            # bf16: fp32 exponent range (quick-max shift can leave exp
            # args ~O(25)) and 16-bit data keeps TensorScalarPtr on the
            # fast DVE uop path (fp32 src is ~20x slower there).
            pun = work.tile([P, MT, SEQ], bf16)    # unnormalized exp
            nrm = work.tile([P, MT, SEQ], bf16)    # normalized staging
            # dual-scalar tensor_scalar (both scalars from SBUF PTR) hits
            # the fast DVE uop; single-AP-scalar + bypass is ~20x slower.
            one = work.tile([P, 1], f32, tag="one", name="one")
            nc.gpsimd.memset(one[:], 1.0)
            negC = [work.tile([P, 1], f32, tag=f"negC{m}", name=f"negC{m}")
                    for m in range(MT)]
            acc = [work.tile([P, ST], f32, tag=f"acc{m}", name=f"acc{m}")
                   for m in range(MT)]
            rinv = [work.tile([P, 1], f32, tag=f"rinv{m}", name=f"rinv{m}")
                    for m in range(MT)]

            def do_mms(m, s, ps2, klo, khi):
                for k in range(klo, khi):
                    nc.tensor.matmul(
                        ps2[:],
                        tmpT_sb[:, k, P * m:P * (m + 1)],
                        id_sb[:, s, k, :],
                        start=(k == 0), stop=(k == KT - 1),
                    )

            def do_post(m, s, ps2):
                if s == 0:
                    nc.vector.tensor_reduce(
                        negC[m][:], ps2[:], axis=AX, op=ALU.max, negate=True)
                nc.scalar.activation(
                    pun[:, m, SB * s:SB * (s + 1)], ps2[:],
                    ACTF.Exp, bias=negC[m][:], scale=1.0,
                    accum_out=acc[m][:, s:s + 1])

            def do_block(m, s):
                ps2 = psp.tile([P, SB], f32, tag="mm2", bufs=4)
                do_mms(m, s, ps2, 0, KT)
                do_post(m, s, ps2)

            def finish(m, engs, dengs):
                stot = work.tile([P, 1], f32, tag=f"stot{m}", name=f"stot{m}")
                nc.vector.reduce_sum(stot[:], acc[m][:], axis=AX)
                nc.vector.reciprocal(rinv[m][:], stot[:])
                # per-chunk normalize -> DMA, fanned across engines/rings
                nchunk = len(engs)
                cw = SEQ // nchunk
                for j in range(nchunk):
                    lo, hi = cw * j, cw * (j + 1)
                    if engs[j] == "scalar_act":
                        nc.scalar.activation(
                            nrm[:, m, lo:hi], pun[:, m, lo:hi],
                            ACTF.Copy, scale=rinv[m][:])
                    else:
                        engs[j].tensor_scalar(
                            nrm[:, m, lo:hi], pun[:, m, lo:hi],
                            rinv[m][:], one[:],
                            op0=ALU.mult, op1=ALU.mult)
                    dengs[j].dma_start(
                        out[P * m:P * (m + 1), lo:hi], nrm[:, m, lo:hi])

            # id even-block descriptors woven in here: each issues after
            # the preceding scalar-engine dispatches, so backpressure on
            # the scalar ring never delays an exp dispatch.
            id_dma(nc.scalar, 0)
            id_dma(nc.scalar, 2)

            # s-interleaved while the id stream is still arriving, then
            # m0 finishes ~9us ahead of m1 so finish(0) fully overlaps.
            for s in range(5):
                do_block(0, s)
                do_block(1, s)
                if s < 2:
                    # s4 is the one scalar-ring block the PE waits on
                    id_dma(nc.scalar, 2 * s + 4, quarters=(s == 0))
            for s in range(5, ST):
                do_block(0, s)
            # m0's finish overlaps m1's matmuls: keep it off the Scalar
            # engine (its FIFO feeds m1's exps -> PSUM bank drains) and
            # off the scalar DMA ring (its sequencer dispatches the exps);
            # both rings are free of input traffic by now.
            # finish(0) stays entirely off the scalar sequencer: even its
            # DIRECT2Ds would wait for the nrm data and stall m1's exps.
            finish(0,
                   engs=[nc.vector, nc.gpsimd, nc.vector, nc.gpsimd],
                   dengs=[nc.sync, nc.sync, nc.sync, nc.sync])
            for s in range(5, ST):
                do_block(1, s)
            # tail: 8 fine chunks, engines interleaved so the first DMAs
            # launch ~0.3us after rinv and both rings stream in parallel
            finish(1,
                   engs=[nc.vector, "scalar_act", nc.vector, nc.gpsimd,
                         nc.vector, "scalar_act", nc.vector, nc.gpsimd],
                   dengs=[nc.sync, nc.scalar, nc.sync, nc.scalar,
                          nc.sync, nc.scalar, nc.sync, nc.scalar])

    nc.compile()
    return nc


def _prepare(user_emb, id_emb, socail_uid_emb, attn_W, w1, w2):
    """Host-side combine + sharding + packing. Returns (0.0, False, in_maps).

    Packed layouts (per-partition contiguous runs -> few big DMA
    descriptors):
      cT:  [128, KT, ROWS]   elem [p,k,m] = c[rows0+m, k*128+p]   (fp16)
      Wp:  [128, KT, HID]    elem [p,h,c] = W[h*128+p, c]         (fp16)
      idT: [ST, 128, KT, SB] elem [s,p,k,c] = id[s*512+c, k*128+p] (fp16)
    """
    w1 = float(np.asarray(w1))
    w2 = float(np.asarray(w2))
    c = (w1 * np.asarray(user_emb, np.float32)
         + w2 * np.asarray(socail_uid_emb, np.float32)).astype(np.float16)

    Wh = np.asarray(attn_W, np.float32).astype(np.float16)
    Wp_pack = np.ascontiguousarray(Wh.reshape(KT, P, HID).transpose(1, 0, 2))

    idh = np.asarray(id_emb, np.float32).astype(np.float16)      # [SEQ, HID]
    idT_pack = np.ascontiguousarray(
        idh.reshape(ST, SB, KT, P).transpose(0, 3, 2, 1)         # [s,p,k,c]
    )

    in_maps = []
    for i in range(NCORES):
        rows = slice(ROWS * i, ROWS * (i + 1))
        cpack = np.ascontiguousarray(
            c[rows].reshape(ROWS, KT, P).transpose(2, 1, 0))
        in_maps.append({
            "cT": cpack,
            "Wp": Wp_pack,
            "idT": idT_pack,
        })
    return 0.0, False, in_maps


def kernel(user_emb, id_emb, socail_uid_emb, attn_W, attn_b, w1, w2):
    from concourse.bass_utils import run_bass_kernel_spmd

    _, _, in_maps = _prepare(
        user_emb, id_emb, socail_uid_emb, attn_W, w1, w2)

    nc = _graph_cache.get("nc")
    if nc is None:
        nc = _build()
        _graph_cache["nc"] = nc

    res = run_bass_kernel_spmd(nc, in_maps, core_ids=list(range(NCORES)))
    return np.concatenate(
        [res.results[i]["out"].astype(np.float32) for i in range(NCORES)], axis=0)


# revision 37
# speedup vs baseline: 1.0206x; 1.0206x over previous
"""Trainium2 8-core kernel for nn_Attn_user_47863115547245.

reference:
    proj     = id_emb @ attn_W.T + attn_b                  # [seq, hid]
    energies = w1*(user @ proj.T) + w2*(socail @ proj.T)   # [state, seq]
    out      = softmax(energies, axis=-1)

Restructuring (exact up to float rounding):
  * linearity: energies = c @ W @ id.T with c = w1*user + w2*socail,
    computed on the host (c is the only state-side tensor uploaded).
  * (c @ W) first: state(2048) < seq(4096) makes this ordering cheaper.
  * the bias term is constant along the softmax axis -> cancels; dropped.

Sharding: data-parallel over state rows, 256 rows/core x 8 cores.
W and id_emb (fp16, pre-transposed/packed) replicated; softmax is
row-local -> zero collectives.

Schedule (from NTFF traces; see kernel body comments):
  * DMA queues: sequencers stall on DIRECT2D queue-depth backpressure,
    so the scalar ring carries only W evens up front (its engine must
    dispatch casts/exps on time) with the id even blocks woven into
    the mm2 emission stream; the sync ring (no engine work) carries
    cT + W odds + id odds + output.
  * warmup garbage matmuls bridge PE from t~7us to the first mm1 dep
    (the HAM clock gate re-throttles after long idle gaps).
  * mm1 single pass, h-arrival paced: h-outer accumulation into 4 PSUM
    banks (2 k-blocks packed per bank = one accumulation group), then
    4 bank copies spread over DVE+ACT so mm2's k-accumulation starts
    without serializing behind one engine queue.
  * mm2 per 512-col seq block, k-inner, 4 rotating PSUM banks; blocks
    s0..s4 alternate m0/m1 (stays just behind the id arrival stream),
    then m0 s5..7, finish(0) fully overlapped with m1 s5..7's matmuls.
  * quick-max softmax: per-row max of bank s0 only is the exp shift
    (gap to the true row max is ~O(10) for gaussian energies, well
    inside fp32 exp range), so there are no per-chunk MAX ops and no
    rescale bookkeeping. Unnormalized exp is held bf16 (fp32 exponent
    range; 16-bit data keeps TensorScalarPtr on the fast DVE uop -
    fp32 src there is ~20x slower); normalize is x(1/S) as a
    dual-AP-scalar tensor_scalar, fanned across DVE/GpSimd/ACT and
    chunk-pipelined with the output DMAs on both rings.
"""

import numpy as np

STATE, SEQ, HID = 2048, 4096, 1024
NCORES = 8
ROWS = STATE // NCORES        # 256 state rows per core
P = 128                       # partitions
KT = HID // P                 # 8 contraction tiles
MT = ROWS // P                # 2 output row tiles per core
SB = 512                      # seq block (one fp32 PSUM bank)
ST = SEQ // SB                # 8 seq blocks
WARMUP_MM = 12                # PE warmup matmuls (HAM un-throttle)

_graph_cache: dict = {}


def _build(*_ignored):
    """Build the per-core Bass graph (no runtime parameters)."""
    import concourse.bacc as bacc
    import concourse.mybir as mybir
    import concourse.bass as bass
    from concourse import tile

    f32, f16 = mybir.dt.float32, mybir.dt.float16
    bf16 = mybir.dt.bfloat16
    AX = mybir.AxisListType.X
    ALU = mybir.AluOpType
    ACTF = mybir.ActivationFunctionType

    nc = bacc.Bacc()

    cT = nc.declare_dram_parameter("cT", [P, KT, ROWS], f16, isOutput=False)
    Wp = nc.declare_dram_parameter("Wp", [P, KT, HID], f16, isOutput=False)
    idT = nc.declare_dram_parameter("idT", [ST, P, KT, SB], f16, isOutput=False)
    out = nc.declare_dram_parameter("out", [ROWS, SEQ], bf16, isOutput=True)

    with tile.TileContext(nc) as tc:
        with (
            tc.tile_pool(name="inp", bufs=1) as inp,
            tc.tile_pool(name="work", bufs=1) as work,
            tc.tile_pool(name="psum", bufs=1,
                         space=bass.MemorySpace.PSUM) as psp,
        ):
            # ---- PE warmup: garbage matmuls lift the HAM clock gate ----
            wgarb = work.tile([P, SB], f16, tag="warmgarb")
            nc.gpsimd.memset(wgarb[:], 0.0)
            # mm1 accumulator: 4 banks, each packing 2 k-blocks of 256
            ps1 = psp.tile([P, 4, 2, ROWS], f32, tag="mm1")
            for _ in range(WARMUP_MM):
                nc.tensor.matmul(
                    ps1[:, 0, 0, :], wgarb[:, :P], wgarb[:, :ROWS],
                    start=True, stop=True)

            # ---- input DMAs, alternating between the two HWDGE rings ----
            cT_sb = inp.tile([P, KT, ROWS], f16)
            W_sb = inp.tile([P, KT, HID], f16)
            id_sb = inp.tile([P, ST, KT, SB], f16)

            # Queue plan (two HWDGE rings, ~188GB/s each): a sequencer
            # stalls on DIRECT2D queue-depth backpressure but NOT on
            # dispatching engine ops into engine queues -- so scalar's
            # engine work (casts, exps) is emitted BEFORE its deep id
            # descriptor list, and the id descriptors are woven into the
            # mm2 emission stream below.
            #   scalar ring: W evens, then id even blocks (woven).
            #   sync ring (no engine work): cT quarters woven with W
            #   odds, then id odd blocks, then the output DMAs.
            # mm1 h-step h needs only W[h] + cT[h] -> h-paced from ~9us.
            H2 = KT // 2

            def id_dma(eng, s, quarters=False):
                # k-split pieces back-to-back on one ring: each piece's
                # completion sem frees its k-range of mm2 matmuls before
                # the whole block lands (quarters for the blocks the PE
                # is known to wait on).
                if quarters:
                    eng.dma_start(id_sb[:, s, 0:2, :], idT[s][:, 0:2, :])
                    eng.dma_start(id_sb[:, s, 2:4, :], idT[s][:, 2:4, :])
                else:
                    eng.dma_start(id_sb[:, s, :H2, :], idT[s][:, :H2, :])
                eng.dma_start(id_sb[:, s, H2:, :], idT[s][:, H2:, :])

            for q in range(4):
                nc.scalar.dma_start(W_sb[:, 2 * q, :], Wp[:, 2 * q, :])
                nc.sync.dma_start(
                    cT_sb[:, 2 * q:2 * q + 2, :], cT[:, 2 * q:2 * q + 2, :])
                nc.sync.dma_start(W_sb[:, 2 * q + 1, :], Wp[:, 2 * q + 1, :])
            for s in range(1, ST, 2):
                id_dma(nc.sync, s)

            # ---- mm1: tmpT[k,m] = sum_h W[h,k]*c[m,h], h-outer --------
            # Single W-arrival-paced pass over all 4 banks.
            # Each bank holds 2 packed k-blocks as ONE accumulation group
            # (start=True zeroes the whole 2KB bank: first write starts
            # the group, last write stops it, pending-zero bits zero each
            # region on first touch).
            tmpT_sb = work.tile([P, KT, ROWS], f16)

            for h in range(KT):
                for kb in range(KT):
                    nc.tensor.matmul(
                        ps1[:, kb // 2, kb % 2, :],
                        W_sb[:, h, P * kb:P * (kb + 1)],
                        cT_sb[:, h, :],
                        start=(h == 0 and kb % 2 == 0),
                        stop=(h == KT - 1 and kb % 2 == 1),
                    )
            # bank copies spread over 3 engines so mm2's k-accumulation
            # isn't serialized behind one DVE queue
            nc.vector.tensor_copy(tmpT_sb[:, 0:2, :], ps1[:, 0, :, :])
            nc.scalar.activation(
                tmpT_sb[:, 2:4, :], ps1[:, 1, :, :], ACTF.Copy)
            nc.scalar.activation(
                tmpT_sb[:, 4:6, :], ps1[:, 2, :, :], ACTF.Copy)
            nc.vector.tensor_copy(tmpT_sb[:, 6:8, :], ps1[:, 3, :, :])

            # ---- mm2 (per seq block, k-inner) + quick-max softmax ----
            # bf16: fp32 exponent range (quick-max shift can leave exp
            # args ~O(25)) and 16-bit data keeps TensorScalarPtr on the
            # fast DVE uop path (fp32 src is ~20x slower there).
            pun = work.tile([P, MT, SEQ], bf16)    # unnormalized exp
            nrm = work.tile([P, MT, SEQ], bf16)    # normalized staging
            # dual-scalar tensor_scalar (both scalars from SBUF PTR) hits
            # the fast DVE uop; single-AP-scalar + bypass is ~20x slower.
            one = work.tile([P, 1], f32, tag="one", name="one")
            nc.gpsimd.memset(one[:], 1.0)
            negC = [work.tile([P, 1], f32, tag=f"negC{m}", name=f"negC{m}")
                    for m in range(MT)]
            acc = [work.tile([P, ST], f32, tag=f"acc{m}", name=f"acc{m}")
                   for m in range(MT)]
            rinv = [work.tile([P, 1], f32, tag=f"rinv{m}", name=f"rinv{m}")
                    for m in range(MT)]

            def do_mms(m, s, ps2, klo, khi):
                for k in range(klo, khi):
                    nc.tensor.matmul(
                        ps2[:],
                        tmpT_sb[:, k, P * m:P * (m + 1)],
                        id_sb[:, s, k, :],
                        start=(k == 0), stop=(k == KT - 1),
                    )

            def do_post(m, s, ps2):
                if s == 0:
                    nc.vector.tensor_reduce(
                        negC[m][:], ps2[:], axis=AX, op=ALU.max, negate=True)
                nc.scalar.activation(
                    pun[:, m, SB * s:SB * (s + 1)], ps2[:],
                    ACTF.Exp, bias=negC[m][:], scale=1.0,
                    accum_out=acc[m][:, s:s + 1])

            def do_block(m, s):
                ps2 = psp.tile([P, SB], f32, tag="mm2", bufs=4)
                do_mms(m, s, ps2, 0, KT)
                do_post(m, s, ps2)

            def finish(m, engs, dengs):
                stot = work.tile([P, 1], f32, tag=f"stot{m}", name=f"stot{m}")
                nc.vector.reduce_sum(stot[:], acc[m][:], axis=AX)
                nc.vector.reciprocal(rinv[m][:], stot[:])
                # per-chunk normalize -> DMA, fanned across engines/rings
                nchunk = len(engs)
                cw = SEQ // nchunk
                for j in range(nchunk):
                    lo, hi = cw * j, cw * (j + 1)
                    if engs[j] == "scalar_act":
                        nc.scalar.activation(
                            nrm[:, m, lo:hi], pun[:, m, lo:hi],
                            ACTF.Copy, scale=rinv[m][:])
                    else:
                        engs[j].tensor_scalar(
                            nrm[:, m, lo:hi], pun[:, m, lo:hi],
                            rinv[m][:], one[:],
                            op0=ALU.mult, op1=ALU.mult)
                    dengs[j].dma_start(
                        out[P * m:P * (m + 1), lo:hi], nrm[:, m, lo:hi])

            # id even-block descriptors woven in here: each issues after
            # the preceding scalar-engine dispatches, so backpressure on
            # the scalar ring never delays an exp dispatch.
            id_dma(nc.scalar, 0)
            id_dma(nc.scalar, 2)

            # s-interleaved while the id stream is still arriving, then
            # m0 finishes ~9us ahead of m1 so finish(0) fully overlaps.
            for s in range(5):
                do_block(0, s)
                do_block(1, s)
                if s < 2:
                    id_dma(nc.scalar, 2 * s + 4)
            for s in range(5, ST):
                do_block(0, s)
            # m0's finish overlaps m1's matmuls: keep it off the Scalar
            # engine (its FIFO feeds m1's exps -> PSUM bank drains) and
            # off the scalar DMA ring (its sequencer dispatches the exps);
            # both rings are free of input traffic by now.
            # finish(0) stays entirely off the scalar sequencer: even its
            # DIRECT2Ds would wait for the nrm data and stall m1's exps.
            finish(0,
                   engs=[nc.vector, nc.gpsimd, nc.vector, nc.gpsimd],
                   dengs=[nc.sync, nc.sync, nc.sync, nc.sync])
            for s in range(5, ST):
                do_block(1, s)
            # tail: 8 fine chunks, engines interleaved so the first DMAs
            # launch ~0.3us after rinv and both rings stream in parallel
            finish(1,
                   engs=[nc.vector, "scalar_act", nc.vector, nc.gpsimd,
                         nc.vector, "scalar_act", nc.vector, nc.gpsimd],
                   dengs=[nc.sync, nc.scalar, nc.sync, nc.scalar,
                          nc.sync, nc.scalar, nc.sync, nc.scalar])

    nc.compile()
    return nc


def _prepare(user_emb, id_emb, socail_uid_emb, attn_W, w1, w2):
    """Host-side combine + sharding + packing. Returns (0.0, False, in_maps).

    Packed layouts (per-partition contiguous runs -> few big DMA
    descriptors):
      cT:  [128, KT, ROWS]   elem [p,k,m] = c[rows0+m, k*128+p]   (fp16)
      Wp:  [128, KT, HID]    elem [p,h,c] = W[h*128+p, c]         (fp16)
      idT: [ST, 128, KT, SB] elem [s,p,k,c] = id[s*512+c, k*128+p] (fp16)
    """
    w1 = float(np.asarray(w1))
    w2 = float(np.asarray(w2))
    c = (w1 * np.asarray(user_emb, np.float32)
         + w2 * np.asarray(socail_uid_emb, np.float32)).astype(np.float16)

    Wh = np.asarray(attn_W, np.float32).astype(np.float16)
    Wp_pack = np.ascontiguousarray(Wh.reshape(KT, P, HID).transpose(1, 0, 2))

    idh = np.asarray(id_emb, np.float32).astype(np.float16)      # [SEQ, HID]
    idT_pack = np.ascontiguousarray(
        idh.reshape(ST, SB, KT, P).transpose(0, 3, 2, 1)         # [s,p,k,c]
    )

    in_maps = []
    for i in range(NCORES):
        rows = slice(ROWS * i, ROWS * (i + 1))
        cpack = np.ascontiguousarray(
            c[rows].reshape(ROWS, KT, P).transpose(2, 1, 0))
        in_maps.append({
            "cT": cpack,
            "Wp": Wp_pack,
            "idT": idT_pack,
        })
    return 0.0, False, in_maps


def kernel(user_emb, id_emb, socail_uid_emb, attn_W, attn_b, w1, w2):
    from concourse.bass_utils import run_bass_kernel_spmd

    _, _, in_maps = _prepare(
        user_emb, id_emb, socail_uid_emb, attn_W, w1, w2)

    nc = _graph_cache.get("nc")
    if nc is None:
        nc = _build()
        _graph_cache["nc"] = nc

    res = run_bass_kernel_spmd(nc, in_maps, core_ids=list(range(NCORES)))
    return np.concatenate(
        [res.results[i]["out"].astype(np.float32) for i in range(NCORES)], axis=0)
